# revision 9
# baseline (speedup 1.0000x reference)
"""Longformer self-attention — nn_LongformerSelfAttention_65687229825616.

kernel(**inputs) takes FULL unsharded inputs (as in setup_inputs) and returns
the FULL (B, T, D) fp32 output. Hardcoded shapes: B=2, T=2048, D=1024, H=16,
hd=64, WINDOW=128, N_GLOBAL=1.

Sharding: data-parallel over (batch, 512-row T-chunk) -> 8 NeuronCores, each
with a 128-row key/value halo. No collectives: the only cross-chunk coupling
is the global row 0 (attends all T keys), handled with per-core partial
softmax sums combined on host (flash-attention style).

Device math (per core, all matmuls bf16 with fp32 PSUM accumulation):
  QT = (Wq.T*s).T @ xq^T + bq*s       [D, 512]   (transposed layout, d on partitions)
  KT = Wk @ xk^T                      [D, 640]   (bk dropped: per-row-constant shift
                                                  Q.bk is softmax-invariant)
  V  = xk @ Wv.T                      [640, D]   (bv dropped: rows of attn sum to 1,
                                                  so bv passes through attention; it is
                                                  folded into bo' = bo + bv@Wo.T)
  per (head, 128-row query block):
    S[q, 0:256]  = Qb.K_window        (one matmul, Q/K transposed tiles)
    S[q, 256]    = Qb.k0              (global key col)
    E = exp(S)                        (ScalarE LUT; no row-max needed, |S| small)
    P, den = exp(S+mask), rowsum      (DVE add; ACT exp with accum_out)
    D = I * (1/den)                   (diagonal matrix of recips)
    PT = P^T scaled by 1/den          (PE "transpose" = matmul with rhs=D)
    OT[d, q] = V_a^T PT_a + V_b^T PT_b + v0^T PTg   (accumulated in PSUM)
  outT = Wo @ attnT + bo'             [D, 512], host transposes back
  row0 partials: s0 = q0.K_own, p0 = exp(s0), num = p0^T V_own, den = sum(p0)
"""
import numpy as np
import ml_dtypes

B, T, D, H, HD, W = 2, 2048, 1024, 16, 64, 128
CH = 512          # query rows per core
HALO = 128
XK = HALO + CH    # 640 key rows per core
NB = CH // 128    # 4 query blocks per core
NCORES = 8
SCALE = np.float32(HD ** -0.5)
BF16 = ml_dtypes.bfloat16

_CACHE = {}


def _build_bass():
    import concourse.bass as bass
    import concourse.bacc as bacc
    import concourse.mybir as mybir
    import concourse.tile as tile
    from contextlib import ExitStack

    f32 = mybir.dt.float32
    bf16 = mybir.dt.bfloat16
    Exp = mybir.ActivationFunctionType.Exp

    nc = bacc.Bacc()
    # ---- DRAM I/O (per-core views; host slices the full tensors) ----
    d_xqT = nc.dram_tensor("xqT", [D, CH], bf16, kind="ExternalInput")
    d_xkT = nc.dram_tensor("xkT", [D, XK], bf16, kind="ExternalInput")
    d_wq = nc.dram_tensor("wq", [D, D], bf16, kind="ExternalInput")   # Wq.T * s
    d_wk = nc.dram_tensor("wk", [D, D], bf16, kind="ExternalInput")   # Wk.T
    d_wv = nc.dram_tensor("wv", [D, D], bf16, kind="ExternalInput")   # Wv.T
    d_wo = nc.dram_tensor("wo", [D, D], bf16, kind="ExternalInput")   # Wo.T
    d_bq = nc.dram_tensor("bq", [128, 8], f32, kind="ExternalInput")  # bq*s packed
    d_bo = nc.dram_tensor("bo", [128, 8], f32, kind="ExternalInput")  # bo+bv@Wo.T packed
    d_q0 = nc.dram_tensor("q0", [128, 8], bf16, kind="ExternalInput")  # (x0@Wq.T+bq)*s
    d_k0 = nc.dram_tensor("k0", [128, 8], bf16, kind="ExternalInput")  # x0@Wk.T
    d_v0 = nc.dram_tensor("v0", [1, D], bf16, kind="ExternalInput")    # x0@Wv.T
    d_msk = nc.dram_tensor("masks", [NB, 128, 257], bf16, kind="ExternalInput")
    d_id = nc.dram_tensor("ident", [128, 128], bf16, kind="ExternalInput")
    d_outT = nc.dram_tensor("outT", [D, CH], f32, kind="ExternalOutput")
    d_r0n = nc.dram_tensor("r0num", [1, H * HD], f32, kind="ExternalOutput")
    d_r0d = nc.dram_tensor("r0den", [4, H], f32, kind="ExternalOutput")

    outT_r = d_outT.rearrange("(n p) m -> n p m", p=128)

    with tile.TileContext(nc) as tc, ExitStack() as ctx:
        consts = ctx.enter_context(tc.tile_pool(name="consts", bufs=1))
        work = ctx.enter_context(tc.tile_pool(name="work", bufs=3))
        # PSUM: 8 banks total; four pools x 2 bufs x 1 bank each.
        pp = ctx.enter_context(tc.tile_pool(name="pp", bufs=2, space="PSUM"))
        ps = ctx.enter_context(tc.tile_pool(name="ps", bufs=2, space="PSUM"))
        ppt = ctx.enter_context(tc.tile_pool(name="ppt", bufs=2, space="PSUM"))
        pot = ctx.enter_context(tc.tile_pool(name="pot", bufs=2, space="PSUM"))

        # ---- load constants / inputs into SBUF ----
        xq = consts.tile([128, 8, CH], bf16)
        xk = consts.tile([128, 8, XK], bf16)
        wq = consts.tile([128, 8, D], bf16)
        wk = consts.tile([128, 8, D], bf16)
        wv = consts.tile([128, 8, D], bf16)
        wo = consts.tile([128, 8, D], bf16)
        msk = consts.tile([128, NB, 257], bf16)
        bq = consts.tile([128, 8], f32)
        bo = consts.tile([128, 8], f32)
        q0 = consts.tile([128, 8], bf16)
        k0 = consts.tile([128, 8], bf16)
        v0 = consts.tile([1, D], bf16)
        ident = consts.tile([128, 128], bf16)

        for (dst, src) in ((xq, d_xqT), (xk, d_xkT), (wq, d_wq), (wk, d_wk),
                           (wv, d_wv), (wo, d_wo)):
            src_r = src.rearrange("(n p) m -> p n m", p=128)
            for i in range(8):
                nc.sync.dma_start(out=dst[:, i, :], in_=src_r[:, i, :])
        nc.sync.dma_start(out=msk[:], in_=d_msk.rearrange("n p m -> p n m"))
        for (dst, src) in ((bq, d_bq), (bo, d_bo), (q0, d_q0), (k0, d_k0),
                           (v0, d_v0), (ident, d_id)):
            nc.sync.dma_start(out=dst[:], in_=src[:])
        ones = consts.tile([128, 1], bf16)
        nc.vector.memset(ones, 1.0)

        # ---- projections ----
        QT = consts.tile([128, 8, CH], bf16)    # [d, t] per 128-row d-chunk
        KT = consts.tile([128, 8, XK], bf16)
        V = consts.tile([128, 5, D], bf16)      # natural [t, d], 5 t-chunks
        attnT = consts.tile([128, 8, CH], bf16)

        for j in range(8):
            qps = pp.tile([128, CH], f32, tag="proj")
            for i in range(8):
                nc.tensor.matmul(qps, wq[:, i, 128 * j:128 * j + 128], xq[:, i, :],
                                 start=(i == 0), stop=(i == 7))
            nc.scalar.activation(QT[:, j, :], qps,
                                 mybir.ActivationFunctionType.Identity,
                                 bias=bq[:, j:j + 1])

        for j in range(8):
            kpsA = pp.tile([128, 512], f32, tag="proj")
            kpsB = pp.tile([128, 128], f32, tag="proj")
            for i in range(8):
                nc.tensor.matmul(kpsA, wk[:, i, 128 * j:128 * j + 128],
                                 xk[:, i, 0:512], start=(i == 0), stop=(i == 7))
            for i in range(8):
                nc.tensor.matmul(kpsB, wk[:, i, 128 * j:128 * j + 128],
                                 xk[:, i, 512:640], start=(i == 0), stop=(i == 7))
            nc.vector.tensor_copy(KT[:, j, 0:512], kpsA)
            nc.vector.tensor_copy(KT[:, j, 512:640], kpsB)

        for t in range(5):
            for dv in range(2):
                vps = pp.tile([128, 512], f32, tag="proj")
                for i in range(8):
                    nc.tensor.matmul(vps, xk[:, i, 128 * t:128 * t + 128],
                                     wv[:, i, 512 * dv:512 * dv + 512],
                                     start=(i == 0), stop=(i == 7))
                nc.vector.tensor_copy(V[:, t, 512 * dv:512 * dv + 512], vps)

        # ---- banded attention: 16 heads x 4 query blocks ----
        for h in range(H):
            po = 64 * (h % 2)          # partition offset of head h in d-chunks
            hj = h // 2
            QTh = QT[po:po + 64, hj, :]
            KTh = KT[po:po + 64, hj, :]
            k0h = k0[po:po + 64, hj:hj + 1]
            for qb in range(NB):
                S = ps.tile([128, 257], f32, tag="s")
                nc.tensor.matmul(S[:, 0:256], QTh[:, 128 * qb:128 * qb + 128],
                                 KTh[:, 128 * qb:128 * qb + 256],
                                 start=True, stop=True)
                nc.tensor.matmul(S[:, 256:257], QTh[:, 128 * qb:128 * qb + 128],
                                 k0h, start=True, stop=True)
                SM = work.tile([128, 257], bf16, tag="e")
                nc.vector.tensor_add(SM, S, msk[:, qb, :])
                P = work.tile([128, 257], bf16, tag="p")
                den = work.tile([128, 1], f32, tag="den")
                nc.scalar.activation(P, SM, Exp, accum_out=den)
                rec = work.tile([128, 1], f32, tag="rec")
                nc.vector.reciprocal(rec, den)
                Dg = work.tile([128, 128], bf16, tag="dg")
                nc.scalar.activation(Dg, ident,
                                     mybir.ActivationFunctionType.Copy,
                                     0.0, rec)
                # PT = diag(1/den) applied during PE transpose: out = P^T scaled
                PT = ppt.tile([128, 384], f32, tag="pt")
                nc.tensor.matmul(PT[:, 0:128], P[:, 0:128], Dg, start=True, stop=True)
                nc.tensor.matmul(PT[:, 128:256], P[:, 128:256], Dg, start=True, stop=True)
                nc.tensor.matmul(PT[0:1, 256:384], P[:, 256:257], Dg, start=True, stop=True)
                PTs = work.tile([128, 256], bf16, tag="pts")
                nc.vector.tensor_copy(PTs, PT[:, 0:256])
                PTg = work.tile([1, 128], bf16, tag="ptg")
                nc.vector.tensor_copy(PTg, PT[0:1, 256:384])
                OT = pot.tile([64, 128], f32, tag="ot")
                nc.tensor.matmul(OT, V[:, qb, 64 * h:64 * h + 64], PTs[:, 0:128],
                                 start=True, stop=False)
                nc.tensor.matmul(OT, V[:, qb + 1, 64 * h:64 * h + 64], PTs[:, 128:256],
                                 start=False, stop=False)
                nc.tensor.matmul(OT, v0[0:1, 64 * h:64 * h + 64], PTg,
                                 start=False, stop=True)
                nc.scalar.copy(attnT[po:po + 64, hj, 128 * qb:128 * qb + 128], OT)

        # ---- global row 0: partial softmax over this core's own 512 keys ----
        r0n = consts.tile([1, H * HD], f32)
        r0d = consts.tile([4, H], f32)
        for h in range(H):
            po = 64 * (h % 2)
            hj = h // 2
            KTh = KT[po:po + 64, hj, :]
            q0h = q0[po:po + 64, hj:hj + 1]
            s0 = pot.tile([128, 4], f32, tag="ot")
            for ck in range(4):
                nc.tensor.matmul(s0[:, ck:ck + 1],
                                 KTh[:, 128 * (ck + 1):128 * (ck + 1) + 128],
                                 q0h, start=True, stop=True)
            p0 = work.tile([128, 4], bf16, tag="p0")
            nc.scalar.activation(p0, s0, Exp)
            dn = ps.tile([4, 1], f32, tag="s")
            nc.tensor.matmul(dn, p0, ones, start=True, stop=True)
            nm = ppt.tile([1, 64], f32, tag="pt")
            for ck in range(4):
                nc.tensor.matmul(nm, p0[:, ck:ck + 1],
                                 V[:, ck + 1, 64 * h:64 * h + 64],
                                 start=(ck == 0), stop=(ck == 3))
            nc.vector.tensor_copy(r0n[0:1, 64 * h:64 * h + 64], nm)
            nc.vector.tensor_copy(r0d[:, h:h + 1], dn)
        nc.sync.dma_start(out=d_r0n[:], in_=r0n[:])
        nc.sync.dma_start(out=d_r0d[:], in_=r0d[:])

        # ---- output projection: outT = Wo @ attnT + bo' ----
        for j in range(8):
            ops = pp.tile([128, CH], f32, tag="proj")
            for i in range(8):
                nc.tensor.matmul(ops, wo[:, i, 128 * j:128 * j + 128],
                                 attnT[:, i, :], start=(i == 0), stop=(i == 7))
            ob = work.tile([128, CH], f32, tag="ob")
            nc.scalar.activation(ob, ops,
                                 mybir.ActivationFunctionType.Identity,
                                 bias=bo[:, j:j + 1])
            nc.sync.dma_start(out=outT_r[j], in_=ob)

    nc.finalize()
    return nc


def _masks():
    """Additive masks (0 / -100) [4, 128, 257] for blocks 0,1,N,N in bf16.

    Col j in [0,256): key t_k = 128*(blk-1)+j ; col 256: global key 0 slot.
    """
    r = np.arange(128)[:, None]
    j = np.arange(257)[None, :]
    band = (j >= r) & (j <= r + 128)
    NEG = np.float32(-100.0)
    maskN = np.where(band | (j == 256), 0.0, NEG).astype(np.float32)
    mask1 = np.where((band | (j == 0)) & (j != 256), 0.0, NEG).astype(np.float32)
    mask0 = np.where((j >= 128) & (j <= r + 128) & (j != 256), 0.0, NEG).astype(np.float32)
    mN = np.stack([maskN] * 4)
    m0 = np.stack([mask0, mask1, maskN, maskN])
    return m0.astype(BF16), mN.astype(BF16)


def _pack(v):  # [1024] -> [128, 8] with col n = v[128n:128n+128]
    return np.ascontiguousarray(v.reshape(8, 128).T)


def _make_in_maps(x, Wq, bq, Wk, bk, Wv, bv, Wo, bo):
    x = np.asarray(x, np.float32)
    Wq, Wk, Wv, Wo = (np.asarray(a, np.float32) for a in (Wq, Wk, Wv, Wo))
    bq, bk, bv, bo = (np.asarray(a, np.float32) for a in (bq, bk, bv, bo))
    wq_h = np.ascontiguousarray((Wq.T * SCALE).astype(BF16))
    wk_h = np.ascontiguousarray(Wk.T.astype(BF16))
    wv_h = np.ascontiguousarray(Wv.T.astype(BF16))
    wo_h = np.ascontiguousarray(Wo.T.astype(BF16))
    bq_h = _pack(bq * SCALE).astype(np.float32)
    bo_h = _pack(bo + bv @ Wo.T).astype(np.float32)
    m0, mN = _masks()
    ident = np.eye(128, dtype=BF16)

    in_maps = []
    for c in range(NCORES):
        b, c4 = divmod(c, 4)
        lo, hi = CH * c4, CH * c4 + CH
        xq_c = x[b, lo:hi]                       # [512, 1024]
        xk_c = np.zeros((XK, D), np.float32)
        klo = max(0, lo - HALO)
        xk_c[HALO - (lo - klo):] = x[b, klo:hi]
        x0 = x[b, 0]
        q0 = (x0 @ Wq.T + bq) * SCALE
        k0 = x0 @ Wk.T
        v0 = x0 @ Wv.T
        in_maps.append({
            "xqT": np.ascontiguousarray(xq_c.T).astype(BF16),
            "xkT": np.ascontiguousarray(xk_c.T).astype(BF16),
            "wq": wq_h, "wk": wk_h, "wv": wv_h, "wo": wo_h,
            "bq": bq_h, "bo": bo_h,
            "q0": _pack(q0).astype(BF16),
            "k0": _pack(k0).astype(BF16),
            "v0": np.ascontiguousarray(v0[None, :]).astype(BF16),
            "masks": m0 if c4 == 0 else mN,
            "ident": ident,
        })

    return in_maps


def kernel(x, Wq, bq, Wk, bk, Wv, bv, Wo, bo):
    from concourse.bass_utils import run_bass_kernel_spmd

    x = np.asarray(x, np.float32)
    Wq, Wk, Wv, Wo = (np.asarray(a, np.float32) for a in (Wq, Wk, Wv, Wo))
    bq, bk, bv, bo = (np.asarray(a, np.float32) for a in (bq, bk, bv, bo))

    if "nc" not in _CACHE:
        _CACHE["nc"] = _build_bass()
    nc = _CACHE["nc"]

    in_maps = _make_in_maps(x, Wq, bq, Wk, bk, Wv, bv, Wo, bo)
    _CACHE["last_in_maps"] = in_maps
    res = run_bass_kernel_spmd(nc, in_maps, core_ids=list(range(NCORES)),
                               **_CACHE.get("run_kwargs", {}))
    _CACHE["last_results"] = res

    out = np.empty((B, T, D), np.float32)
    for c in range(NCORES):
        b, c4 = divmod(c, 4)
        out[b, CH * c4:CH * c4 + CH] = res.results[c]["outT"].T
    # host fix-up for global row 0 (full softmax across all T keys)
    for b in range(B):
        num = np.zeros((H, HD), np.float64)
        den = np.zeros((H,), np.float64)
        for c in range(4 * b, 4 * b + 4):
            num += res.results[c]["r0num"].reshape(H, HD)
            den += res.results[c]["r0den"].sum(axis=0)
        attn0 = (num / den[:, None]).reshape(D).astype(np.float32) + bv
        out[b, 0] = attn0 @ Wo.T + bo
    return out
